# revision 1
# baseline (speedup 1.0000x reference)
"""PatchCore greedy-coreset selection on 8 Trainium2 NeuronCores.

Strategy: project features on-device (fp32 PE matmuls), AllGather the
projected bank, run the 163-step farthest-point-sampling loop in the
squared-distance domain with a per-step (value, index) AllGather for the
global argmax.  The distance matrix is never materialized: anchor0 has a
closed form and each step needs only one matvec against the local shard.
Returns features[selected_indices] (host-side row gather of the input)."""
import sys
sys.path.insert(0, "/opt/trn_rl_repo")
import numpy as np
import concourse.bass as bass
import concourse.mybir as mybir

F32 = mybir.dt.float32
U32 = mybir.dt.uint32
N = 16384
DIN = 1024
DP = 128
NSTEPS = 163
NCORES = 8
SH = N // NCORES          # 2048 rows per core
NCL = SH // 128           # 16 chunks
BIG = 1.0e30


class Ser:
    """Serial-chain semaphore tracker: every instruction waits on the
    previous instruction's completion marker."""

    def __init__(self, nc, ctx):
        self.nc = nc
        self.sems = {}
        self.counts = {}
        for name in ("pe", "dve", "act", "pool", "sp", "dma", "cc"):
            self.sems[name] = ctx.enter_context(nc.semaphore(f"ser_{name}"))
            self.counts[name] = 0
        self.last = None  # (sem_name, value)

    def wait(self, eng):
        if self.last is not None:
            sem_name, val = self.last
            eng.wait_ge(self.sems[sem_name], val)

    def mark(self, inst, kind, inc=None):
        if inc is None:
            inc = 16 if kind == "dma" else 1
        inst.then_inc(self.sems[kind], inc)
        self.counts[kind] += inc
        self.last = (kind, self.counts[kind])

    # combined helpers -------------------------------------------------
    def dma(self, eng, dst, src):
        self.wait(eng)
        i = eng.dma_start(dst, src)
        self.mark(i, "dma")
        return i

    def op(self, eng_name, inst_fn):
        """inst_fn(engine) -> instruction; waits before, marks after."""
        eng = getattr(self.nc, {"pe": "tensor", "dve": "vector",
                                "act": "scalar", "pool": "gpsimd",
                                "sp": "sync"}[eng_name])
        self.wait(eng)
        inst = inst_fn(eng)
        self.mark(inst, eng_name)
        return inst


def build(nc=None):
    if nc is None:
        nc = bass.Bass()
    from contextlib import ExitStack
    ctx = ExitStack()

    # ---- I/O ----
    ft = nc.dram_tensor("ft", [DIN, SH], F32, kind="ExternalInput")
    w = nc.dram_tensor("w", [DIN, DP], F32, kind="ExternalInput")
    pcon = nc.dram_tensor("pcon", [128, 1], F32, kind="ExternalInput")
    ident = nc.dram_tensor("ident", [128, 128], F32, kind="ExternalInput")
    sel = nc.dram_tensor("sel", [1, NSTEPS], F32, kind="ExternalOutput")

    # collective bounce buffers (internal DRAM)
    ag1_in = nc.dram_tensor("ag1_in", [128, SH], F32)
    ag1_out = nc.dram_tensor("ag1_out", [NCORES * 128, SH], F32, addr_space="Shared")
    ag2_in = nc.dram_tensor("ag2_in", [1, SH], F32)
    ag2_out = nc.dram_tensor("ag2_out", [NCORES, SH], F32, addr_space="Shared")
    cc_in = nc.dram_tensor("cc_in", [1, 2], F32)
    cc_out = nc.dram_tensor("cc_out", [NCORES, 2], F32, addr_space="Shared")

    rg = [list(range(NCORES))]

    s = Ser(nc, ctx)
    sb = lambda shape, dt=F32: ctx.enter_context(nc.sbuf_tensor(shape, dt))
    ps = lambda shape, dt=F32: ctx.enter_context(nc.psum_tensor(shape, dt))

    # ---- SBUF ----
    w_sb = sb([128, 8 * DP])          # weight chunks: [:, kc*128:(kc+1)*128]
    ft_sb = sb([128, 8 * SH])         # featT chunks: [:, kc*SH + n]
    rts_sb = sb([128, SH])            # reducedT shard
    r2_sb = sb([128, SH])             # squared shard
    rtf_sb = sb([128, N])             # full reducedT
    sqg_sb = sb([1, N])               # full sq row
    sqrow_sb = sb([1, SH])            # local sq row
    sqtile_sb = sb([128, NCL])        # local sq in (p, cl) layout
    anchor_sb = sb([128, NCL])
    pcon_sb = sb([128, 1])
    ident_sb = sb([128, 128])
    ones_col = sb([128, 1])
    ones_row = sb([1, 128])
    s2_col = sb([128, 1])             # -2 * sum_j r_j
    Sb_col = sb([128, 1])             # S broadcast over partitions
    S_sb = sb([1, 1])
    vals8 = sb([128, 8])
    idx8 = sb([128, 8], U32)
    gidx = sb([128, 1])
    pack = sb([128, 2])
    tsb = sb([2, 128])
    cm = sb([1, 8])
    meq = sb([1, 128])
    mbig = sb([1, 128])
    msel = sb([1, 128])
    jloc = sb([1, 1])
    pack2 = sb([1, 2])
    g16 = sb([1, 16])
    gm = sb([1, 8])
    geq = sb([1, 8])
    gbig = sb([1, 8])
    gsel = sb([1, 8])
    jg = sb([1, 1])
    jgu = sb([1, 1], U32)
    vneg = sb([128, 1])
    sqjrep = sb([1, NCL])
    t2_sb = sb([128, NCL])
    selrow = sb([1, NSTEPS])

    psum_pr = ps([128, 512])
    psum_sq = ps([1, SH])
    psum_mv = ps([128, NCL])
    psum_t = ps([2, 128])
    psum_b = ps([128, 1])

    TS = mybir.AluOpType
    AX = mybir.AxisListType

    # =================== precompute ===================
    s.dma(nc.sync, w_sb[:], w.rearrange("(a p) m -> p (a m)", p=128))
    s.dma(nc.sync, ft_sb[:], ft.rearrange("(a p) n -> p (a n)", p=128))
    s.dma(nc.sync, pcon_sb[:], pcon[:])
    s.dma(nc.sync, ident_sb[:], ident[:])
    s.op("dve", lambda e: e.memset(ones_col[:], 1.0))
    s.op("dve", lambda e: e.memset(ones_row[:], 1.0))

    # projection: reducedT = W.T @ featT  (per 512-wide ntile, 8 k-chunks)
    for nt in range(SH // 512):
        s.wait(nc.tensor)
        for kc in range(8):
            i = nc.tensor.matmul(
                psum_pr[:, :],
                w_sb[:, kc * DP:(kc + 1) * DP],
                ft_sb[:, kc * SH + nt * 512: kc * SH + (nt + 1) * 512],
                start=(kc == 0), stop=(kc == 7),
            )
        s.mark(i, "pe")
        s.op("dve", lambda e: e.tensor_copy(
            rts_sb[:, nt * 512:(nt + 1) * 512], psum_pr[:, :]))

    # r2 = rts^2 ; sq_row = ones.T @ r2
    s.op("dve", lambda e: e.tensor_tensor(r2_sb[:], rts_sb[:], rts_sb[:], TS.mult))
    s.wait(nc.tensor)
    for nt in range(SH // 512):
        i = nc.tensor.matmul(
            psum_sq[0:1, nt * 512:(nt + 1) * 512],
            ones_col[:], r2_sb[:, nt * 512:(nt + 1) * 512],
            start=True, stop=True)
    s.mark(i, "pe")
    s.op("dve", lambda e: e.tensor_copy(sqrow_sb[:], psum_sq[:]))

    # sq_tile[p, cl] = sq_local[cl*128 + p]
    s.dma(nc.sync, sqtile_sb[:],
          sqrow_sb.rearrange("o (cl p) -> p (o cl)", p=128))

    # AllGather reducedT and sq
    s.dma(nc.gpsimd, ag1_in[:], rts_sb[:])
    s.wait(nc.gpsimd)
    i = nc.gpsimd.collective_compute(
        "AllGather", TS.bypass, replica_groups=rg,
        ins=[ag1_in[:]], outs=[ag1_out[:]])
    s.mark(i, "cc")
    s.dma(nc.gpsimd, ag2_in[:], sqrow_sb[:])
    s.wait(nc.gpsimd)
    i = nc.gpsimd.collective_compute(
        "AllGather", TS.bypass, replica_groups=rg,
        ins=[ag2_in[:]], outs=[ag2_out[:]])
    s.mark(i, "cc")
    s.dma(nc.sync, rtf_sb[:], ag1_out.rearrange("(c p) n -> p (c n)", p=128))
    s.dma(nc.sync, sqg_sb[:], ag2_out.rearrange("c n -> (c n)")[None, :])

    # s = rowsum(rtf); s2 = -2 s ; S = sum(sqg); Sb = bcast(S)
    s.op("dve", lambda e: e.tensor_reduce(s2_col[:], rtf_sb[:], AX.X, TS.add))
    s.op("dve", lambda e: e.tensor_scalar(s2_col[:], s2_col[:], -2.0, None, TS.mult))
    s.op("dve", lambda e: e.tensor_reduce(S_sb[:], sqg_sb[:], AX.X, TS.add))
    s.op("pe", lambda e: e.matmul(psum_b[:], ones_row[:], S_sb[:],
                                  start=True, stop=True))
    s.op("dve", lambda e: e.tensor_copy(Sb_col[:], psum_b[:]))

    # anchor2_0 = (16384*sq_tile + Sb) + (-2 r.s)
    s.wait(nc.tensor)
    for cl in range(NCL):
        i = nc.tensor.matmul(
            psum_mv[:, cl:cl + 1],
            rts_sb[:, cl * 128:(cl + 1) * 128], s2_col[:],
            start=True, stop=True)
    s.mark(i, "pe")
    s.op("dve", lambda e: e.tensor_scalar(
        t2_sb[:], sqtile_sb[:], float(N), Sb_col[:, 0:1], TS.mult, TS.add))
    s.op("dve", lambda e: e.tensor_tensor(
        anchor_sb[:], t2_sb[:], psum_mv[:], TS.add))

    # =================== loop ===================
    jreg = ctx.enter_context(nc.vector.register("jreg"))

    for t in range(NSTEPS):
        # local argmax
        s.op("dve", lambda e: e.max(vals8[:], anchor_sb[:]))
        s.op("dve", lambda e: e.max_index(idx8[:], vals8[:], anchor_sb[:]))
        s.op("dve", lambda e: e.tensor_copy(gidx[:], idx8[:, 0:1]))
        s.op("dve", lambda e: e.tensor_scalar(
            gidx[:], gidx[:], 128.0, pcon_sb[:, 0:1], TS.mult, TS.add))
        s.op("dve", lambda e: e.tensor_copy(pack[:, 0:1], vals8[:, 0:1]))
        s.op("dve", lambda e: e.tensor_copy(pack[:, 1:2], gidx[:]))
        s.op("pe", lambda e: e.transpose(psum_t[:], pack[:], ident_sb[:]))
        s.op("dve", lambda e: e.tensor_copy(tsb[:], psum_t[:]))
        # core winner
        s.op("dve", lambda e: e.max(cm[:], tsb[0:1, :]))
        s.op("dve", lambda e: e.tensor_scalar(
            meq[:], tsb[0:1, :], cm[0:1, 0:1], None, TS.is_equal))
        s.op("dve", lambda e: e.tensor_scalar(
            mbig[:], meq[:], -BIG, BIG, TS.mult, TS.add))
        s.op("dve", lambda e: e.tensor_tensor(
            msel[:], meq[:], tsb[1:2, :], TS.mult))
        s.op("dve", lambda e: e.tensor_tensor(
            msel[:], msel[:], mbig[:], TS.add))
        s.op("dve", lambda e: e.tensor_reduce(jloc[:], msel[:], AX.X, TS.min))
        s.op("dve", lambda e: e.tensor_copy(pack2[:, 0:1], cm[:, 0:1]))
        s.op("dve", lambda e: e.tensor_copy(pack2[:, 1:2], jloc[:]))
        # exchange top-1s
        s.dma(nc.gpsimd, cc_in[:], pack2[:])
        s.wait(nc.gpsimd)
        i = nc.gpsimd.collective_compute(
            "AllGather", TS.bypass, replica_groups=rg,
            ins=[cc_in[:]], outs=[cc_out[:]])
        s.mark(i, "cc")
        s.dma(nc.gpsimd, g16[:], cc_out.rearrange("c n -> (c n)")[None, :])
        # global winner (vals at even positions, gidx at odd)
        gv = g16.rearrange("o (c two) -> o c two", two=2)
        s.op("dve", lambda e: e.max(gm[:], gv[:, :, 0]))
        s.op("dve", lambda e: e.tensor_scalar(
            geq[:], gv[:, :, 0], gm[0:1, 0:1], None, TS.is_equal))
        s.op("dve", lambda e: e.tensor_scalar(
            gbig[:], geq[:], -BIG, BIG, TS.mult, TS.add))
        s.op("dve", lambda e: e.tensor_tensor(
            gsel[:], geq[:], gv[:, :, 1], TS.mult))
        s.op("dve", lambda e: e.tensor_tensor(
            gsel[:], gsel[:], gbig[:], TS.add))
        s.op("dve", lambda e: e.tensor_reduce(jg[:], gsel[:], AX.X, TS.min))
        s.op("dve", lambda e: e.tensor_copy(selrow[0:1, t:t + 1], jg[:]))
        s.op("dve", lambda e: e.tensor_copy(jgu[:], jg[:]))
        # dynamic index -> register
        s.op("dve", lambda e: e.reg_load(jreg, jgu[0:1, 0:1]))
        jv = nc.vector.snap(jreg, min_val=0, max_val=N - 1)
        # v = -2 * reducedT[:, j] ; sqjrep = sq[j] * ones
        s.op("dve", lambda e: e.tensor_scalar(
            vneg[:], rtf_sb[:, bass.ds(jv, 1)], -2.0, None, TS.mult))
        s.op("dve", lambda e: e.tensor_scalar(
            sqjrep[:], ones_row[0:1, 0:NCL], sqg_sb[0:1, bass.ds(jv, 1)],
            None, TS.mult))
        # matvec + sq_j fold
        s.wait(nc.tensor)
        for cl in range(NCL):
            nc.tensor.matmul(
                psum_mv[:, cl:cl + 1],
                rts_sb[:, cl * 128:(cl + 1) * 128], vneg[:],
                start=True, stop=False)
        i = nc.tensor.matmul(psum_mv[:, :], ones_row[:], sqjrep[:],
                             start=False, stop=True)
        s.mark(i, "pe")
        # anchor2 = min(anchor2, psum + sq_tile)
        s.op("dve", lambda e: e.tensor_tensor(
            t2_sb[:], psum_mv[:], sqtile_sb[:], TS.add))
        s.op("dve", lambda e: e.tensor_tensor(
            anchor_sb[:], anchor_sb[:], t2_sb[:], TS.min))

    s.dma(nc.sync, sel[:], selrow[:])
    # keep allocations alive until compile; Bass keeps its own refs
    nc._pc_ctx = ctx
    return nc


def make_in_maps(features, W):
    featT = np.ascontiguousarray(features.T)  # [1024, 16384]
    in_maps = []
    for c in range(NCORES):
        pcon = (c * SH + np.arange(128, dtype=np.float32)).reshape(128, 1)
        in_maps.append({
            "ft": np.ascontiguousarray(featT[:, c * SH:(c + 1) * SH]),
            "w": np.ascontiguousarray(W),
            "pcon": pcon,
            "ident": np.eye(128, dtype=np.float32),
        })
    return in_maps


_CACHE = {}


def _run(in_maps):
    from concourse.bass_utils import run_bass_kernel_spmd
    if "nc" not in _CACHE:
        _CACHE["nc"] = build()
    res = run_bass_kernel_spmd(_CACHE["nc"], in_maps, list(range(NCORES)),
                               trace=TRACE)
    return res


TRACE = False
LAST_EXEC_NS = None


def kernel(features: np.ndarray, W: np.ndarray) -> np.ndarray:
    global LAST_EXEC_NS
    features = np.ascontiguousarray(features, dtype=np.float32)
    W = np.ascontiguousarray(W, dtype=np.float32)
    assert features.shape == (N, DIN) and W.shape == (DIN, DP)
    res = _run(make_in_maps(features, W))
    LAST_EXEC_NS = res.exec_time_ns
    sel = res.results[0]["sel"].reshape(-1)
    idx = sel.astype(np.int64)
    return features[idx]
